# revision 17
# baseline (speedup 1.0000x reference)
"""Dir_Encoder_GCN (2-layer GCNConv + ELU + Softplus) on 8 trn2 NeuronCores.

Strategy (per sharding hint): nodes are dst-sharded across 8 cores; edges are
partitioned by destination shard and sorted by destination. Weights W1/W2 are
replicated. Source-feature tables (dinv-scaled) are exchanged via AllGather.

Math: for each layer, out[d] = dinv[d] * sum_e w_e * (dinv[s_e] * feat[s_e]) + b
with self-loops folded in as ordinary edges of weight 1. The per-edge gather is
an indirect DMA of table rows; the weighted segmented sum is a PE matmul with a
host-prepared scaled one-hot matrix M (M[p, slot_p] = w_p); W is applied after
aggregation via pre^T @ W per 128-slot window (linearity of the aggregation).

Host-side numpy performs only integer index manipulation and data layout
(sorting, window packing, one-hot placement of input edge weights); all
floating-point arithmetic on values happens on-device.
"""

import sys

if "/opt/trn_rl_repo" not in sys.path:
    sys.path.insert(0, "/opt/trn_rl_repo")

import numpy as np

N_NODES = 50000
N_EDGES = 800000
F_IN = 128
F_HID = 128
F_OUT = 64
NCORES = 8
P = 128  # partitions / window slot capacity / edge-tile size


def _pack_windows(dst_local, tw_cap, n_nodes_core):
    """Greedy-pack local nodes into windows of <=128 nodes and <=tw_cap*128
    edges. dst_local: sorted local dst id per edge. Returns list of
    (node_lo, node_hi) per window (node_hi exclusive)."""
    # edges per local node
    counts = np.bincount(dst_local, minlength=n_nodes_core)
    windows = []
    lo = 0
    cur_edges = 0
    hi = 0
    cap = tw_cap * P
    while hi < n_nodes_core:
        c = counts[hi]  # node's total edge count (self-loop included)
        if (hi - lo) >= P or cur_edges + c > cap:
            windows.append((lo, hi))
            lo = hi
            cur_edges = 0
        cur_edges += c
        hi += 1
    windows.append((lo, hi))
    return windows, counts


def build_problem(x, edge_index, edge_weight, W1, b1, W2, b2):
    """Builds the bass program + per-core input maps.

    Returns (nc, in_maps, row_of_node): run the program SPMD on cores 0..7,
    concat the per-core "y_win" outputs, then index with row_of_node to get
    the final [N, F_OUT] output.
    """
    import concourse.bacc as bacc
    import concourse.tile as tile
    from concourse import bass, mybir

    x = np.asarray(x, dtype=np.float32)
    edge_index = np.asarray(edge_index)
    edge_weight = np.asarray(edge_weight, dtype=np.float32)
    W1 = np.asarray(W1, dtype=np.float32)
    b1 = np.asarray(b1, dtype=np.float32)
    W2 = np.asarray(W2, dtype=np.float32)
    b2 = np.asarray(b2, dtype=np.float32)

    n = x.shape[0]
    F_IN = x.shape[1]
    F_HID = W1.shape[1]
    F_OUT = W2.shape[1]

    # ---------------- host-side integer prep ----------------
    # nodes -> cores, contiguous ranges
    per_core_n = (n + NCORES - 1) // NCORES  # 6250
    src = edge_index[0].astype(np.int64)
    dst = edge_index[1].astype(np.int64)

    # fold self-loops in as ordinary edges (weight 1, matching reference)
    src_all = np.concatenate([src, np.arange(n, dtype=np.int64)])
    dst_all = np.concatenate([dst, np.arange(n, dtype=np.int64)])
    w_all = np.concatenate([edge_weight, np.ones(n, dtype=np.float32)])

    order = np.argsort(dst_all, kind="stable")
    s_s = src_all[order]
    d_s = dst_all[order]
    w_s = w_all[order]

    core_edge_bounds = np.searchsorted(
        d_s, [c * per_core_n for c in range(NCORES + 1)]
    )

    # edge tiles per window (17*128 = 2176 edge capacity); raised if any
    # single node's edge count exceeds one window's capacity
    TW = max(17, int(np.ceil((np.bincount(dst_all, minlength=n).max() + 1) / P)))

    # pass 1: per-core window packing to find uniform NWIN
    core_data = []
    nwin_max = 0
    kd_max = 1
    for c in range(NCORES):
        e0, e1 = core_edge_bounds[c], core_edge_bounds[c + 1]
        n_lo = c * per_core_n
        n_hi = min((c + 1) * per_core_n, n)
        n_c = n_hi - n_lo
        dl = (d_s[e0:e1] - n_lo).astype(np.int64)
        windows, counts = _pack_windows(dl, TW, n_c)
        nwin_max = max(nwin_max, len(windows))
        # real (non-self-loop) in-degree for ELL: counts includes self-loop
        kd_max = max(kd_max, int((counts - 1).max(initial=0)))
        core_data.append((e0, e1, n_lo, n_c, dl, windows, counts))

    NWIN = nwin_max
    KD = kd_max
    SH = NWIN * P  # table rows per core (window-slot layout)
    TTOT = NWIN * TW  # edge tiles per core per layer
    VTOT = NCORES * SH

    # pass 2: build per-core arrays
    in_maps = []
    row_of_node = np.zeros(n, dtype=np.int64)  # global table row per node
    node_rows_per_core = []

    # first compute row_of_node for ALL cores (needed for gidx of any core)
    for c in range(NCORES):
        e0, e1, n_lo, n_c, dl, windows, counts = core_data[c]
        rows = np.full(SH, -1, dtype=np.int64)  # local row -> node id
        for wi, (lo, hi) in enumerate(windows):
            ids = np.arange(lo, hi)
            row_of_node[n_lo + ids] = c * SH + wi * P + (ids - lo)
            rows[wi * P : wi * P + (hi - lo)] = n_lo + ids
        node_rows_per_core.append(rows)

    for c in range(NCORES):
        e0, e1, n_lo, n_c, dl, windows, counts = core_data[c]
        e_src = s_s[e0:e1]
        e_w = w_s[e0:e1]
        # edge order is dst-sorted; windows take contiguous edge runs
        node_e0 = np.concatenate([[0], np.cumsum(counts)])  # per local node

        gidx = np.zeros((P, TTOT), dtype=np.int32)
        m_host = np.zeros((TTOT * P, P), dtype=np.float32)
        # window-slot layout inputs
        x_win = np.zeros((SH, F_IN), dtype=np.float32)
        wdeg = np.zeros((P, NWIN * KD), dtype=np.float32)

        for wi, (lo, hi) in enumerate(windows):
            ew0, ew1 = node_e0[lo], node_e0[hi]
            cnt = ew1 - ew0
            assert cnt <= TW * P
            wsrc = e_src[ew0:ew1]
            wslot = dl[ew0:ew1] - lo
            ww = e_w[ew0:ew1]
            t0 = wi * TW
            # scatter edges into tiles: edge j -> tile t0 + j//P, partition j%P
            tt = t0 + np.arange(cnt) // P
            pp = np.arange(cnt) % P
            gidx[pp, tt] = row_of_node[wsrc].astype(np.int32)
            m_host[tt * P + pp, wslot] = ww

            # window-slot node data
            ids = np.arange(lo, hi)
            x_win[wi * P : wi * P + (hi - lo)] = x[n_lo + ids]
            # ELL of real in-edge weights (exclude self-loop weight):
            # edges of node v: e_src slice; self-loop is the one with src==v
            # and w==1 appended last among its dst group (stable sort kept
            # original order: real edges first, then self-loop)
            for v in ids:
                a, b = node_e0[v], node_e0[v + 1]
                # last entry for node v is its self-loop (appended after real
                # edges and stable-sorted)
                realw = e_w[a : b - 1]
                p_ = v - lo
                wdeg[p_, wi * KD : wi * KD + len(realw)] = realw

        in_maps.append(
            {
                "x_win": x_win,
                "wdeg": wdeg,
                "gidx": gidx,
                "m_stream": m_host,
                "w1": W1,
                "w2": W2,
                "b1b": np.tile(b1[None, :], (P, 1)).astype(np.float32),
                "b2b": np.tile(b2[None, :], (P, 1)).astype(np.float32),
                "ident": np.eye(P, dtype=np.float32),
            }
        )

    # ---------------- device program (uniform across cores) ----------------
    import os

    stage = int(os.environ.get("GCN_BUILD_STAGE", "9"))  # debug bisection
    nc = bacc.Bacc("TRN2", target_bir_lowering=False, debug=False, num_devices=NCORES)

    x_win_d = nc.dram_tensor("x_win", [SH, F_IN], mybir.dt.float32, kind="ExternalInput")
    wdeg_d = nc.dram_tensor("wdeg", [P, NWIN * KD], mybir.dt.float32, kind="ExternalInput")
    gidx_d = nc.dram_tensor("gidx", [P, TTOT], mybir.dt.int32, kind="ExternalInput")
    m_d = nc.dram_tensor("m_stream", [TTOT * P, P], mybir.dt.float32, kind="ExternalInput")
    w1_d = nc.dram_tensor("w1", [F_IN, F_HID], mybir.dt.float32, kind="ExternalInput")
    w2_d = nc.dram_tensor("w2", [F_HID, F_OUT], mybir.dt.float32, kind="ExternalInput")
    b1b_d = nc.dram_tensor("b1b", [P, F_HID], mybir.dt.float32, kind="ExternalInput")
    b2b_d = nc.dram_tensor("b2b", [P, F_OUT], mybir.dt.float32, kind="ExternalInput")
    ident_d = nc.dram_tensor("ident", [P, P], mybir.dt.float32, kind="ExternalInput")
    y_d = nc.dram_tensor("y_win", [SH, F_OUT], mybir.dt.float32, kind="ExternalOutput")

    AF = mybir.ActivationFunctionType
    OP = mybir.AluOpType

    with tile.TileContext(nc) as tc:
        with (
            tc.tile_pool(name="const", bufs=1) as cpool,
            tc.tile_pool(name="gpool", bufs=24) as gpool,
            tc.tile_pool(name="mpool", bufs=2) as mpool,
            tc.tile_pool(name="post", bufs=3) as post,
            tc.tile_pool(name="pacc", bufs=2, space="PSUM") as pacc,
            tc.tile_pool(name="pmisc", bufs=2, space="PSUM") as pmisc,
            tc.tile_pool(name="dram", bufs=1, space="DRAM") as dpool,
        ):
            # constants
            w1_t = cpool.tile([F_IN, F_HID], mybir.dt.float32)
            nc.sync.dma_start(out=w1_t[:], in_=w1_d[:])
            w2_t = cpool.tile([F_HID, F_OUT], mybir.dt.float32)
            nc.sync.dma_start(out=w2_t[:], in_=w2_d[:])
            b1b_t = cpool.tile([P, F_HID], mybir.dt.float32)
            nc.sync.dma_start(out=b1b_t[:], in_=b1b_d[:])
            b2b_t = cpool.tile([P, F_OUT], mybir.dt.float32)
            nc.sync.dma_start(out=b2b_t[:], in_=b2b_d[:])
            ident_t = cpool.tile([P, P], mybir.dt.float32)
            nc.sync.dma_start(out=ident_t[:], in_=ident_d[:])
            gidx_t = cpool.tile([P, TTOT], mybir.dt.int32)
            nc.sync.dma_start(out=gidx_t[:], in_=gidx_d[:])

            # ---- degree -> dinv [P, NWIN] ----
            wdeg_t = cpool.tile([P, NWIN * KD], mybir.dt.float32)
            nc.sync.dma_start(out=wdeg_t[:], in_=wdeg_d[:])
            dsum_t = cpool.tile([P, NWIN], mybir.dt.float32)
            nc.vector.tensor_reduce(
                out=dsum_t[:],
                in_=wdeg_t[:].rearrange("p (w k) -> p w k", k=KD),
                axis=mybir.AxisListType.X,
                op=OP.add,
            )
            # deg = sum + 1 (self-loop); dinv = sqrt(1/deg)
            recip_t = cpool.tile([P, NWIN], mybir.dt.float32)
            nc.vector.tensor_scalar_add(out=dsum_t[:], in0=dsum_t[:], scalar1=1.0)
            nc.vector.reciprocal(out=recip_t[:], in_=dsum_t[:])
            dinv_t = cpool.tile([P, NWIN], mybir.dt.float32)
            nc.scalar.activation(out=dinv_t[:], in_=recip_t[:], func=AF.Sqrt)

            # ---- xsc table build + allgather ----
            def dummy_out():
                dummy = post.tile([P, F_OUT], mybir.dt.float32, tag="yf")
                nc.scalar.activation(out=dummy[:], in_=b2b_t[:], func=AF.Copy)
                nc.sync.dma_start(out=y_d[0:P, :], in_=dummy[:])

            xsc_shard = dpool.tile([SH, F_IN], mybir.dt.float32)
            xsc_full = dpool.tile([VTOT, F_IN], mybir.dt.float32, addr_space="Shared")
            for wi in range(NWIN):
                xw_t = post.tile([P, F_IN], mybir.dt.float32, tag="xw")
                nc.sync.dma_start(
                    out=xw_t[:], in_=x_win_d[wi * P : (wi + 1) * P, :]
                )
                xs_t = post.tile([P, F_IN], mybir.dt.float32, tag="xs")
                nc.vector.tensor_scalar(
                    out=xs_t[:], in0=xw_t[:],
                    scalar1=dinv_t[:, wi : wi + 1], scalar2=None, op0=OP.mult,
                )
                nc.sync.dma_start(
                    out=xsc_shard[wi * P : (wi + 1) * P, :], in_=xs_t[:]
                )
            if stage >= 2:
                nc.gpsimd.collective_compute(
                    "AllGather",
                    OP.bypass,
                    replica_groups=[list(range(NCORES))],
                    ins=[xsc_shard.opt()],
                    outs=[xsc_full.opt()],
                )

            hs_shard = dpool.tile([SH, F_HID], mybir.dt.float32)
            hs_full = dpool.tile([VTOT, F_HID], mybir.dt.float32, addr_space="Shared")

            def layer(table_full, fdim, w_t, bb_t, out_write):
                """One GCN layer. out_write(wi, tile[P, fout]) stores result."""
                for wi in range(NWIN):
                    mwin_t = mpool.tile([P, TW * P], mybir.dt.float32, tag="mwin")
                    nc.sync.dma_start(
                        out=mwin_t[:].rearrange("p (t s) -> p t s", s=P),
                        in_=m_d[wi * TW * P : (wi + 1) * TW * P, :].rearrange(
                            "(t p) s -> p t s", p=P
                        ),
                    )
                    acc = pacc.tile([P, fdim], mybir.dt.float32, space="PSUM", tag="acc")
                    for t in range(TW):
                        ti = wi * TW + t
                        g_t = gpool.tile([P, fdim], mybir.dt.float32, tag="g")
                        nc.gpsimd.indirect_dma_start(
                            out=g_t[:],
                            out_offset=None,
                            in_=table_full.opt(),
                            in_offset=bass.IndirectOffsetOnAxis(
                                ap=gidx_t[:, ti : ti + 1], axis=0
                            ),
                        )
                        nc.tensor.matmul(
                            out=acc[:],
                            lhsT=mwin_t[:, t * P : (t + 1) * P],
                            rhs=g_t[:],
                            start=(t == 0),
                            stop=(t == TW - 1),
                        )
                    # pre = acc * dinv (per-slot) ; transpose; @W ; activations
                    pre_t = post.tile([P, fdim], mybir.dt.float32, tag="pre")
                    nc.vector.tensor_scalar(
                        out=pre_t[:], in0=acc[:],
                        scalar1=dinv_t[:, wi : wi + 1], scalar2=None, op0=OP.mult,
                    )
                    preT_ps = pmisc.tile([P, fdim], mybir.dt.float32, space="PSUM", tag="preT")
                    nc.tensor.transpose(
                        out=preT_ps[:], in_=pre_t[:], identity=ident_t[:]
                    )
                    preT_t = post.tile([P, fdim], mybir.dt.float32, tag="preT_sb")
                    nc.scalar.copy(out=preT_t[:], in_=preT_ps[:])
                    fout = w_t.shape[1]
                    h_ps = pmisc.tile([P, fout], mybir.dt.float32, space="PSUM", tag="h")
                    nc.tensor.matmul(
                        out=h_ps[:], lhsT=preT_t[:], rhs=w_t[:],
                        start=True, stop=True,
                    )
                    out_write(wi, h_ps)

            # ---- layer 1: table xsc, act = elu, write hs ----
            def l1_out(wi, h_ps):
                hb_t = post.tile([P, F_HID], mybir.dt.float32, tag="hb")
                nc.vector.tensor_add(out=hb_t[:], in0=h_ps[:], in1=b1b_t[:])
                mn_t = post.tile([P, F_HID], mybir.dt.float32, tag="mn")
                nc.vector.tensor_scalar_min(out=mn_t[:], in0=hb_t[:], scalar1=0.0)
                ex_t = post.tile([P, F_HID], mybir.dt.float32, tag="ex")
                nc.scalar.activation(out=ex_t[:], in_=mn_t[:], func=AF.Exp)
                rl_t = post.tile([P, F_HID], mybir.dt.float32, tag="rl")
                nc.vector.tensor_scalar_max(out=rl_t[:], in0=hb_t[:], scalar1=0.0)
                h_t = post.tile([P, F_HID], mybir.dt.float32, tag="hf")
                # (relu - 1) + exp(min(x,0)) = elu
                nc.vector.scalar_tensor_tensor(
                    out=h_t[:], in0=rl_t[:], scalar=-1.0, in1=ex_t[:],
                    op0=OP.add, op1=OP.add,
                )
                hsv_t = post.tile([P, F_HID], mybir.dt.float32, tag="hsv")
                nc.vector.tensor_scalar(
                    out=hsv_t[:], in0=h_t[:],
                    scalar1=dinv_t[:, wi : wi + 1], scalar2=None, op0=OP.mult,
                )
                nc.sync.dma_start(
                    out=hs_shard[wi * P : (wi + 1) * P, :], in_=hsv_t[:]
                )

            if stage >= 3:
                layer(xsc_full, F_IN, w1_t, b1b_t, l1_out)

            if stage >= 4:
                nc.gpsimd.collective_compute(
                    "AllGather",
                    OP.bypass,
                    replica_groups=[list(range(NCORES))],
                    ins=[hs_shard.opt()],
                    outs=[hs_full.opt()],
                )

            # ---- layer 2: table hs, act = softplus + 1e-4 ----
            def l2_out(wi, y_ps):
                # softplus(x) = max(x,0) + ln(1 + exp(-|x|)), then + 1e-4
                yb_t = post.tile([P, F_OUT], mybir.dt.float32, tag="yb")
                nc.vector.tensor_add(out=yb_t[:], in0=y_ps[:], in1=b2b_t[:])
                na_t = post.tile([P, F_OUT], mybir.dt.float32, tag="na")
                nc.vector.scalar_tensor_tensor(
                    out=na_t[:], in0=yb_t[:], scalar=-1.0, in1=yb_t[:],
                    op0=OP.mult, op1=OP.min,
                )
                ex2_t = post.tile([P, F_OUT], mybir.dt.float32, tag="ex2")
                nc.scalar.activation(out=ex2_t[:], in_=na_t[:], func=AF.Exp)
                ln_t = post.tile([P, F_OUT], mybir.dt.float32, tag="ln")
                nc.scalar.activation(out=ln_t[:], in_=ex2_t[:], func=AF.Ln, bias=1.0)
                sp_t = post.tile([P, F_OUT], mybir.dt.float32, tag="sp")
                nc.vector.scalar_tensor_tensor(
                    out=sp_t[:], in0=yb_t[:], scalar=0.0, in1=ln_t[:],
                    op0=OP.max, op1=OP.add,
                )
                yf_t = post.tile([P, F_OUT], mybir.dt.float32, tag="yf")
                nc.vector.tensor_scalar_add(out=yf_t[:], in0=sp_t[:], scalar1=1e-4)
                nc.sync.dma_start(
                    out=y_d[wi * P : (wi + 1) * P, :], in_=yf_t[:]
                )

            if stage >= 5:
                layer(hs_full, F_HID, w2_t, b2b_t, l2_out)
            else:
                dummy_out()

    nc.compile()
    return nc, in_maps, row_of_node


def kernel(x, edge_index, edge_weight, W1, b1, W2, b2):
    from concourse.bass_utils import run_bass_kernel_spmd

    nc, in_maps, row_of_node = build_problem(
        x, edge_index, edge_weight, W1, b1, W2, b2
    )
    res = run_bass_kernel_spmd(nc, in_maps, core_ids=list(range(NCORES)))
    y_full = np.concatenate([res.results[c]["y_win"] for c in range(NCORES)], axis=0)
    out = y_full[row_of_node]  # [n, F_OUT]
    return out.astype(np.float32)


if __name__ == "__main__":
    # quick shape smoke with random data
    rng = np.random.default_rng(0)
    x = rng.standard_normal((N_NODES, F_IN)).astype(np.float32)
    ei = rng.integers(0, N_NODES, size=(2, N_EDGES)).astype(np.int64)
    ew = rng.random(N_EDGES).astype(np.float32)
    W1 = rng.standard_normal((F_IN, F_HID)).astype(np.float32) * 0.09
    W2 = rng.standard_normal((F_HID, F_OUT)).astype(np.float32) * 0.09
    y = kernel(x, ei, ew, W1, np.zeros(F_HID, np.float32), W2, np.zeros(F_OUT, np.float32))
    print(y.shape, y.dtype, np.isfinite(y).all())
